# revision 2
# baseline (speedup 1.0000x reference)
"""Trainium2 Bass kernel for nn_AttentionLayer (Bahdanau-style attention scorer).

Math (per batch b):
    x   = concat([a, broadcast(s)], -1)            # [Tx, Da+Ds]
    h   = relu(x @ W1 + b1)                        # [Tx, H]
    e   = tanh(h @ W2 + b2)                        # [Tx, 1]
    al  = softmax(e, axis=Tx)
    ctx = al^T @ a                                 # [1, Da]

Since e = tanh(.) is in [-1, 1], softmax needs no max subtraction.

Sharding: data-parallel over B across 8 cores (8 batches each).

v4 design — tail-free streaming, 12.75 MB/core:
`a` ships transposed+bf16 (aT, features on partitions) for ALL batches,
plus natural-layout (a_nat) bf16 for batches 4-7 only, queued after the
aT stream so each a_nat lands just-in-time for its ctx matmuls.

Scores: mm1 as column-tiled pairs (two 512-wide time slices through
array cols 0-63/64-127); relu + s-term bias on ACT.  mm2 writes e rows
at PSUM partitions {0,32,64,96} via tile_position; softmax groups are
pairs {0,1} (rows 0/32), {2,3} (rows 64/96) sharing one PSUM tile, and
a page {4,5,6,7} (rows 0/32/64/96).  tanh/exp on ACT with accum_out
partial denominators (summed on host; division on host).

Context:
  - batches 0-3 ("vector route"): PE broadcasts the batch's softmax row
    into PSUM quarters (K=1 matmul with a ones stationary); DVE runs the
    fused scalar_tensor_tensor (mult+accum_out) against aT slices — one
    pass, no separate reduce.  Partial sums per quarter, summed on host.
  - batches 4-7 ("PE route"): p rows gathered to contiguous partitions
    via a scalar-engine DMA (partition-strided SBUF->SBUF), transposed
    time-major by PE (16 transposes for all 4 batches at once), then
    ctx_j = sum_n pT_n^T @ a_nat_n as a single accumulation chain per
    batch at PSUM partition 0.

Host-side preprocessing (transpose/cast/shard + final division) is numpy.
"""

import os
import sys

import numpy as np

for _p in ("/opt/trn_rl_repo", "/root/.axon_site/_ro/trn_rl_repo"):
    if os.path.isdir(_p) and _p not in sys.path:
        sys.path.insert(0, _p)

import ml_dtypes  # noqa: E402

import concourse.bacc as bacc  # noqa: E402
import concourse.bass as bass  # noqa: E402
import concourse.mybir as mybir  # noqa: E402
import concourse.tile as tile  # noqa: E402

BF16 = mybir.dt.bfloat16
F32 = mybir.dt.float32
NPBF16 = ml_dtypes.bfloat16
AF = mybir.ActivationFunctionType
ALU = mybir.AluOpType
PSUM = bass.MemorySpace.PSUM

NCORES = 8
B, TX, DA, DS, H = 64, 2048, 256, 256, 50
BPC = B // NCORES  # batches per core
NT = TX // 128  # 128-wide time chunks (PE-route ctx)
KD = DA // 128  # feature chunks of a

NVEC = 4  # batches 0..NVEC-1 vector route; batches NVEC..7 PE route
KNAT = BPC - NVEC
NQ = 4  # ctx partial quarters per (batch, k) on the vector route
# e-row placement: batch -> (group index, PSUM partition row)
GRP_OF = {0: (0, 0), 1: (0, 32), 2: (1, 64), 3: (1, 96),
          4: (2, 0), 5: (2, 32), 6: (2, 64), 7: (2, 96)}
NGRP = 3
WARMUP = 20


def build_nc():
    """Build the (SPMD-identical) single-core Bass program."""
    nc = bacc.Bacc(
        "TRN2", target_bir_lowering=False, debug=False, num_devices=NCORES
    )

    aT = nc.dram_tensor("aT", [BPC, 128, KD, TX], BF16, kind="ExternalInput")
    a_nat = nc.dram_tensor(
        "a_nat", [KNAT, 128, NT, DA], BF16, kind="ExternalInput"
    )
    w1a = nc.dram_tensor("w1a", [128, KD, 64], BF16, kind="ExternalInput")
    w1s = nc.dram_tensor("w1s", [128, KD, H], F32, kind="ExternalInput")
    sT = nc.dram_tensor("sT", [128, KD, BPC], F32, kind="ExternalInput")
    # b1c / w2c carry two copies of their payload: partition rows 0-49 and
    # 64-113 (the two tile_position row groups used below).
    b1c = nc.dram_tensor("b1c", [128, 1], F32, kind="ExternalInput")
    w2c = nc.dram_tensor("w2c", [128, 32], BF16, kind="ExternalInput")
    b2c = nc.dram_tensor("b2c", [128, 1], F32, kind="ExternalInput")
    ones = nc.dram_tensor("ones", [128, 128], BF16, kind="ExternalInput")
    idm4 = nc.dram_tensor("idm4", [4, 4], BF16, kind="ExternalInput")
    ctxp_o = nc.dram_tensor(
        "ctxp_o", [128, NVEC * KD * NQ], F32, kind="ExternalOutput"
    )
    ctxq_o = nc.dram_tensor("ctxq_o", [1, KNAT, DA], F32, kind="ExternalOutput")
    den_o = nc.dram_tensor("den_o", [128, NGRP, 2], F32, kind="ExternalOutput")

    with tile.TileContext(nc) as tc:
        with tc.tile_pool(name="const", bufs=1) as cpool, tc.tile_pool(
            name="atp", bufs=BPC
        ) as atpool, tc.tile_pool(
            name="anp", bufs=KNAT
        ) as anpool, tc.tile_pool(name="sb2", bufs=1) as sb2:
            # DMA issue order is the schedule (one HWDGE FIFO on Sync):
            # aT tiles stream first in batch order, split in time-halves so
            # mm1 chases each half; a_nat tiles queue after the whole aT
            # stream (their consumers run last).
            at_tiles = []
            for b in range(BPC):
                at_tiles.append(
                    atpool.tile([128, KD, TX], BF16, name=f"at{b}", tag="at")
                )
            an_tiles = []
            for i in range(KNAT):
                an_tiles.append(
                    anpool.tile([128, NT, DA], BF16, name=f"an{i}", tag="an")
                )

            nc.sync.dma_start(at_tiles[0][:, :, 0:1024], aT[0, :, :, 0:1024])

            w1a_sb = cpool.tile([128, KD, 64], BF16)
            nc.gpsimd.dma_start(w1a_sb[:], w1a[:])
            w1s_sb = cpool.tile([128, KD, H], F32)
            nc.gpsimd.dma_start(w1s_sb[:], w1s[:])
            sT_sb = cpool.tile([128, KD, BPC], F32)
            nc.gpsimd.dma_start(sT_sb[:], sT[:])
            b1c_sb = cpool.tile([128, 1], F32)
            nc.gpsimd.dma_start(b1c_sb[:], b1c[:])
            w2c_sb = cpool.tile([128, 32], BF16)
            nc.gpsimd.dma_start(w2c_sb[:], w2c[:])
            b2c_sb = cpool.tile([128, 1], F32)
            nc.gpsimd.dma_start(b2c_sb[:], b2c[:])
            ones_sb = cpool.tile([128, 128], BF16)
            nc.gpsimd.dma_start(ones_sb[:], ones[:])
            idm4_sb = cpool.tile([4, 4], BF16)
            nc.gpsimd.dma_start(idm4_sb[:], idm4[:])

            nc.sync.dma_start(at_tiles[0][:, :, 1024:2048], aT[0, :, :, 1024:2048])
            for b in range(1, BPC):
                nc.sync.dma_start(at_tiles[b][:, :, 0:1024], aT[b, :, :, 0:1024])
                nc.sync.dma_start(
                    at_tiles[b][:, :, 1024:2048], aT[b, :, :, 1024:2048]
                )
            for i in range(KNAT):
                nc.sync.dma_start(an_tiles[i][:, 0:8, :], a_nat[i, :, 0:8, :])
                nc.sync.dma_start(an_tiles[i][:, 8:16, :], a_nat[i, :, 8:16, :])

            sterm_sb = sb2.tile([128, BPC], F32)
            p_ab = sb2.tile([128, TX], BF16)  # softmax rows, batches 0-3
            p_p4 = sb2.tile([128, TX], BF16)  # softmax rows, batches 4-7
            pcomp = sb2.tile([4, TX], BF16)  # gathered p rows (batches 4-7)
            pT_sb = sb2.tile([128, NT, KNAT], BF16)
            sttout = sb2.tile([128, 512], BF16)  # stt elementwise dump
            ctxp_sb = sb2.tile([128, NVEC * KD * NQ], F32)
            ctxq_sb = sb2.tile([1, KNAT, DA], F32)
            den_sb = sb2.tile([128, NGRP, 2], F32)

            with tc.tile_pool(name="hps", bufs=2, space=PSUM) as hps, tc.tile_pool(
                name="eps", bufs=1, space=PSUM
            ) as eps, tc.tile_pool(
                name="pbc", bufs=2, space=PSUM
            ) as pbcp, tc.tile_pool(name="hsb", bufs=6) as hsbp, tc.tile_pool(
                name="tsb", bufs=2
            ) as tsbp:
                # PE warm-up: dense dummy matmuls on zeroed scratch keep the
                # PE busy during the initial DMA window (HAM p-state ramp).
                warm_sb = sb2.tile([128, 512], BF16, tag="warm")
                nc.vector.memset(warm_sb[:], 0.0)
                warm_ps = hps.tile([128, 512], F32, tag="hps", name="warm_ps")
                for _ in range(WARMUP):
                    nc.tensor.matmul(
                        warm_ps[0:64, :],
                        warm_sb[:, 0:64],
                        warm_sb[:],
                        start=True,
                        stop=True,
                        skip_group_check=True,
                    )
                # s-term, twice: partitions 0-49 (row group 0) and 64-113
                # (row group 64), so both relu halves get a bias.
                nc.gpsimd.memset(sterm_sb[:], 0.0)
                nc.gpsimd.memset(den_sb[:], 0.0)
                sterm_ps = hps.tile([128, BPC], F32, tag="hps")
                for cg in (0, 64):
                    for k in range(KD):
                        nc.tensor.matmul(
                            sterm_ps[cg : cg + H, :],
                            w1s_sb[:, k, :],
                            sT_sb[:, k, :],
                            start=(k == 0),
                            stop=(k == KD - 1),
                            tile_position=(0, cg),
                            skip_group_check=True,
                        )
                    nc.scalar.activation(
                        sterm_sb[cg : cg + H, :],
                        sterm_ps[cg : cg + H, :],
                        AF.Identity,
                        bias=b1c_sb[cg : cg + H, :],
                    )

                # FIFO of deferred PE emitters spliced into later PE stream.
                pending = []

                def drain(n):
                    for _ in range(n):
                        if not pending:
                            return
                        pending.pop(0)()

                def emit_vec(b, q, row, p_tile):
                    """Vector-route ctx quarter q for batch b: PE-broadcast
                    p[row, 512q:512q+512] into PSUM, DVE fused mult+accum."""

                    def emit():
                        pb = pbcp.tile([128, 512], F32, tag="pbc", name="pb")
                        nc.tensor.matmul(
                            pb[:],
                            ones_sb[row : row + 1, :],
                            p_tile[row : row + 1, 512 * q : 512 * (q + 1)],
                            start=True,
                            stop=True,
                            tile_position=(row, 0),
                            skip_group_check=True,
                        )
                        for k in range(KD):
                            nc.vector.scalar_tensor_tensor(
                                out=sttout[:],
                                in0=at_tiles[b][:, k, 512 * q : 512 * (q + 1)],
                                scalar=1.0,
                                in1=pb[:],
                                op0=ALU.mult,
                                op1=ALU.mult,
                                accum_out=ctxp_sb[
                                    :, (b * KD + k) * NQ + q : (b * KD + k) * NQ + q + 1
                                ],
                            )

                    return emit

                def emit_softmax(e_t, p0, nrows, gi, hf, p_tile):
                    t_sb = tsbp.tile([128, 1024], F32, tag="tsb")
                    nc.scalar.activation(
                        t_sb[p0 : p0 + nrows, :],
                        e_t[p0 : p0 + nrows, hf, :],
                        AF.Tanh,
                        bias=b2c_sb[p0 : p0 + nrows, :],
                    )
                    nc.scalar.activation(
                        p_tile[p0 : p0 + nrows, 1024 * hf : 1024 * (hf + 1)],
                        t_sb[p0 : p0 + nrows, :],
                        AF.Exp,
                        accum_out=den_sb[p0 : p0 + nrows, gi, hf : hf + 1],
                    )

                def emit_ptq(qgrp):
                    """Transpose p chunks 4*qgrp..4*qgrp+3 for all KNAT
                    batches at once (pcomp rows 0-3 are time-contiguous)."""

                    def emit():
                        pt_ps = pbcp.tile(
                            [128, 4, KNAT], BF16, tag="pbc", name="pt"
                        )
                        for c in range(4):
                            n = 4 * qgrp + c
                            nc.tensor.transpose(
                                pt_ps[:, c, :],
                                pcomp[0:KNAT, 128 * n : 128 * (n + 1)],
                                idm4_sb[0:KNAT, 0:KNAT],
                            )
                        nc.vector.tensor_copy(
                            pT_sb[:, 4 * qgrp : 4 * qgrp + 4, :], pt_ps[:]
                        )

                    return emit

                def emit_quads(j, c_all, nlo, nhi):
                    def emit():
                        for n in range(nlo, nhi):
                            nc.tensor.matmul(
                                c_all[0:1, j, :],
                                pT_sb[:, n, j : j + 1],
                                an_tiles[j][:, n, :],
                                start=(n == 0),
                                stop=(n == NT - 1),
                                tile_position=(0, 0),
                                skip_group_check=True,
                            )

                    return emit

                e_ab = eps.tile([128, 2, 1024], F32, tag="eps", name="e_ab")
                e_p4 = None
                c_all = None
                h_tiles = {}
                for b in range(BPC):
                    at_t = at_tiles[b]
                    if b == NVEC:
                        e_p4 = eps.tile([128, 2, 1024], F32, tag="eps", name="e_p4")
                    gi, row = GRP_OF[b]
                    e_t = e_ab if b < NVEC else e_p4
                    for tp in range(2):
                        h_ps = hps.tile([128, 512], F32, tag="hps")
                        for k in range(KD):
                            for half, cg in enumerate((0, 64)):
                                ts = 2 * tp + half
                                nc.tensor.matmul(
                                    h_ps[cg : cg + 64, :],
                                    w1a_sb[:, k, :],
                                    at_t[:, k, ts * 512 : (ts + 1) * 512],
                                    start=(k == 0),
                                    stop=(k == KD - 1),
                                    tile_position=(0, cg),
                                    skip_group_check=True,
                                )
                        h_sb = hsbp.tile([128, 512], BF16, tag="hsb")
                        h_tiles[(b, tp)] = h_sb
                        nc.scalar.activation(
                            h_sb[:], h_ps[:], AF.Relu, bias=sterm_sb[:, b : b + 1]
                        )
                        drain(2)
                    # mm2: e row for batch b at partition `row`, free layout
                    # [hf, 512*half + u] == global t = 1024*hf + 512*half + u.
                    for hf in range(2):
                        for half, cg in enumerate((0, 64)):
                            nc.tensor.matmul(
                                e_t[row : row + 32, hf, 512 * half : 512 * (half + 1)],
                                w2c_sb[cg : cg + H, :],
                                h_tiles[(b, hf)][cg : cg + H, :],
                                start=True,
                                stop=True,
                                tile_position=(cg, row),
                                skip_group_check=True,
                            )
                    drain(1)
                    if b == 1 or b == 3:
                        p0 = 0 if b == 1 else 64
                        for hf in range(2):
                            emit_softmax(e_ab, p0, 64, gi, hf, p_ab)
                        for bb in (b - 1, b):
                            _, rr = GRP_OF[bb]
                            for q in range(NQ):
                                pending.append(emit_vec(bb, q, rr, p_ab))
                    if b == BPC - 1:
                        for hf in range(2):
                            emit_softmax(e_p4, 0, 128, 2, hf, p_p4)
                        # gather p rows {0,32,64,96} -> contiguous partitions
                        # (scalar-engine HWDGE: separate FIFO from Sync).
                        nc.scalar.dma_start(pcomp[0:KNAT, :], p_p4[0:128:32, :])
                        c_all = eps.tile([1, KNAT, DA], F32, tag="eps", name="c_all")
                        for qgrp in range(4):
                            pending.append(emit_ptq(qgrp))
                        for j in range(KNAT):
                            for nlo in range(0, NT, 4):
                                pending.append(
                                    emit_quads(j, c_all, nlo, nlo + 4)
                                )

                drain(len(pending))
                # vec-route partials + denominators out (SWDGE, idle queue)
                nc.gpsimd.dma_start(ctxp_o[:], ctxp_sb[:])
                nc.gpsimd.dma_start(den_o[:], den_sb[:])
                # PE-route ctx: evacuate the accumulation row and ship last.
                nc.vector.tensor_copy(ctxq_sb[0:1, 0:2, :], c_all[0:1, 0:2, :])
                nc.scalar.activation(
                    ctxq_sb[0:1, 2:KNAT, :], c_all[0:1, 2:KNAT, :], AF.Identity
                )
                nc.sync.dma_start(ctxq_o[:], ctxq_sb[:])

    nc.compile()
    return nc


def make_in_maps(a, s, W1, b1, W2, b2):
    a = np.asarray(a, np.float32)
    s = np.asarray(s, np.float32)
    W1 = np.asarray(W1, np.float32)
    b1 = np.asarray(b1, np.float32)
    W2 = np.asarray(W2, np.float32)
    b2 = np.asarray(b2, np.float32)

    a5 = a.reshape(NCORES, BPC, TX, DA)
    s3 = s.reshape(NCORES, BPC, DS)

    w1a_h = np.zeros((128, KD, 64), np.float32)
    w1a_h[:, :, :H] = W1[:DA].reshape(KD, 128, H).transpose(1, 0, 2)
    w1a_h = w1a_h.astype(NPBF16)
    w1s_h = np.ascontiguousarray(
        W1[DA:].reshape(KD, 128, H).transpose(1, 0, 2)
    ).astype(np.float32)
    b1c_h = np.zeros((128, 1), np.float32)
    b1c_h[0:H, 0] = b1
    b1c_h[64 : 64 + H, 0] = b1
    w2c_h = np.zeros((128, 32), np.float32)
    w2c_h[0:H, 0] = W2[:, 0]
    w2c_h[64 : 64 + H, 0] = W2[:, 0]
    w2c_h = w2c_h.astype(NPBF16)
    b2c_h = np.full((128, 1), float(b2.reshape(-1)[0]), np.float32)
    ones_h = np.ones((128, 128), NPBF16)
    idm4_h = np.eye(4).astype(NPBF16)

    in_maps = []
    for i in range(NCORES):
        ai = a5[i]
        aT_h = np.ascontiguousarray(
            ai.transpose(0, 2, 1)
            .reshape(BPC, KD, 128, TX)
            .transpose(0, 2, 1, 3)
        ).astype(NPBF16)
        a_nat_h = np.ascontiguousarray(
            ai[NVEC:].reshape(KNAT, NT, 128, DA).transpose(0, 2, 1, 3)
        ).astype(NPBF16)
        sT_h = np.ascontiguousarray(
            s3[i].T.reshape(KD, 128, BPC).transpose(1, 0, 2)
        ).astype(np.float32)
        in_maps.append(
            {
                "aT": aT_h,
                "a_nat": a_nat_h,
                "w1a": w1a_h,
                "w1s": w1s_h,
                "sT": sT_h,
                "b1c": b1c_h,
                "w2c": w2c_h,
                "b2c": b2c_h,
                "ones": ones_h,
                "idm4": idm4_h,
            }
        )
    return in_maps


def assemble_output(results):
    outs = []
    for i in range(NCORES):
        r = results[i]
        ctxp = r["ctxp_o"].astype(np.float64).reshape(128, NVEC, KD, NQ)
        ctxq = r["ctxq_o"].astype(np.float64)  # [1, KNAT, DA]
        den3 = r["den_o"].astype(np.float64)  # [128, NGRP, 2]
        full = np.empty((BPC, DA), np.float64)
        for b in range(NVEC):
            full[b] = ctxp[:, b, :, :].sum(-1).T.reshape(DA)
        full[NVEC:] = ctxq[0]
        den = np.empty((BPC, 1), np.float64)
        for b in range(BPC):
            gi, row = GRP_OF[b]
            den[b, 0] = den3[row, gi, :].sum()
        outs.append(full / den)
    return np.concatenate(outs, 0).reshape(B, 1, DA).astype(np.float32)


_NC_CACHE = None


def _get_nc():
    global _NC_CACHE
    if _NC_CACHE is None:
        _NC_CACHE = build_nc()
    return _NC_CACHE


def kernel(a, s, W1, b1, W2, b2, trace=False):
    from concourse.bass_utils import run_bass_kernel_spmd

    nc = _get_nc()
    in_maps = make_in_maps(a, s, W1, b1, W2, b2)
    res = run_bass_kernel_spmd(
        nc, in_maps, core_ids=list(range(NCORES)), trace=trace
    )
    out = assemble_output(res.results)
    if trace:
        kernel.last_exec_time_ns = res.exec_time_ns
        kernel.last_results = res
    return out


# revision 3
# speedup vs baseline: 1.0150x; 1.0150x over previous
"""Trainium2 Bass kernel for nn_AttentionLayer (Bahdanau-style attention scorer).

Math (per batch b):
    x   = concat([a, broadcast(s)], -1)            # [Tx, Da+Ds]
    h   = relu(x @ W1 + b1)                        # [Tx, H]
    e   = tanh(h @ W2 + b2)                        # [Tx, 1]
    al  = softmax(e, axis=Tx)
    ctx = al^T @ a                                 # [1, Da]

Since e = tanh(.) is in [-1, 1], softmax needs no max subtraction.

Sharding: data-parallel over B across 8 cores (8 batches each).

v5 design — tail-free streaming, 12.75 MB/core, diagonal-packed mm2:
`a` ships transposed+bf16 (aT, features on partitions) for ALL batches,
plus natural-layout (a_nat) bf16 for batches 4-7 only, queued after the
aT stream so each a_nat lands just-in-time for its ctx matmuls.

Scores: mm1 as column-tiled pairs (two 512-wide time slices through
array cols 0-63/64-127); relu + s-term bias on ACT.  mm2 uses a
block-diagonal stationary [128, 2] (W2 copy at rows 0-49 -> col 0, rows
64-113 -> col 1), so ONE 512-col matmul per (batch, tp) yields both
time slices: e rows land at PSUM partitions {R, R+1}, R = 32*(b%4).
e/p free layout is [tp, u]: global t = 1024*tp + 512*parity(row) + u.
Softmax groups: pairs {0,1} (rows 0-31..), {2,3} (rows 64..) sharing one
PSUM tile, and a page {4,5,6,7}.  tanh/exp on ACT with accum_out
partial denominators (summed on host; division on host).

Context:
  - batches 0-3 ("vector route"): PE broadcasts the batch's softmax row
    into PSUM quarters (ones-stationary matmul; odd-parity rows are
    first DMA-gathered to 32-aligned partitions since matmul operand
    base partitions must be 32-aligned); DVE runs the fused
    scalar_tensor_tensor (mult + accum_out) against aT slices.
  - batches 4-7 ("PE route"): p rows gathered to contiguous partitions
    0-3 per parity (scalar-engine HWDGE, partition-strided SBUF->SBUF),
    transposed time-major by PE (16 transposes cover all 4 batches),
    then ctx_j = sum_n pT_n^T @ a_nat_n as one accumulation chain per
    batch at PSUM partition 0.

HAM: the PE clock-gate un-throttles only under dense array activity, so
during the early DMA-chase phase (no deferred work available yet) the
drain() scheduler emits 1-column-stationary dummy matmuls to keep the
array streaming at full clock.

Host-side preprocessing (transpose/cast/shard + final division) is numpy.
"""

import os
import sys

import numpy as np

for _p in ("/opt/trn_rl_repo", "/root/.axon_site/_ro/trn_rl_repo"):
    if os.path.isdir(_p) and _p not in sys.path:
        sys.path.insert(0, _p)

import ml_dtypes  # noqa: E402

import concourse.bacc as bacc  # noqa: E402
import concourse.bass as bass  # noqa: E402
import concourse.mybir as mybir  # noqa: E402
import concourse.tile as tile  # noqa: E402

BF16 = mybir.dt.bfloat16
F32 = mybir.dt.float32
NPBF16 = ml_dtypes.bfloat16
AF = mybir.ActivationFunctionType
ALU = mybir.AluOpType
PSUM = bass.MemorySpace.PSUM

NCORES = 8
B, TX, DA, DS, H = 64, 2048, 256, 256, 50
BPC = B // NCORES  # batches per core
NT = TX // 128  # 128-wide time chunks (PE-route ctx)
KD = DA // 128  # feature chunks of a

NVEC = 4  # batches 0..NVEC-1 vector route; batches NVEC..7 PE route
KNAT = BPC - NVEC
NQ = 4  # ctx partial quarters per (batch, k) on the vector route
NGRP = 3  # softmax groups: {0,1}, {2,3}, {4,5,6,7}
WARMUP = 20
DUMMY_MAX = 40


def build_nc():
    """Build the (SPMD-identical) single-core Bass program."""
    nc = bacc.Bacc(
        "TRN2", target_bir_lowering=False, debug=False, num_devices=NCORES
    )

    aT = nc.dram_tensor("aT", [BPC, 128, KD, TX], BF16, kind="ExternalInput")
    a_nat = nc.dram_tensor(
        "a_nat", [KNAT, 128, NT, DA], BF16, kind="ExternalInput"
    )
    w1a = nc.dram_tensor("w1a", [128, KD, 64], BF16, kind="ExternalInput")
    w1s = nc.dram_tensor("w1s", [128, KD, H], F32, kind="ExternalInput")
    sT = nc.dram_tensor("sT", [128, KD, BPC], F32, kind="ExternalInput")
    # b1c carries the bias at partition rows 0-49 and 64-113 (the two
    # relu halves); w2d is the block-diagonal mm2 stationary.
    b1c = nc.dram_tensor("b1c", [128, 1], F32, kind="ExternalInput")
    w2d = nc.dram_tensor("w2d", [128, 2], BF16, kind="ExternalInput")
    b2c = nc.dram_tensor("b2c", [128, 1], F32, kind="ExternalInput")
    ones = nc.dram_tensor("ones", [128, 128], BF16, kind="ExternalInput")
    idm4 = nc.dram_tensor("idm4", [4, 4], BF16, kind="ExternalInput")
    ctxp_o = nc.dram_tensor(
        "ctxp_o", [128, NVEC * KD * NQ], F32, kind="ExternalOutput"
    )
    ctxq_o = nc.dram_tensor("ctxq_o", [1, KNAT, DA], F32, kind="ExternalOutput")
    den_o = nc.dram_tensor("den_o", [128, NGRP, 2], F32, kind="ExternalOutput")

    with tile.TileContext(nc) as tc:
        with tc.tile_pool(name="const", bufs=1) as cpool, tc.tile_pool(
            name="atp", bufs=BPC
        ) as atpool, tc.tile_pool(
            name="anp", bufs=KNAT
        ) as anpool, tc.tile_pool(name="sb2", bufs=1) as sb2:
            # DMA issue order is the schedule (one HWDGE FIFO on Sync):
            # aT tiles stream first in batch order, split in time-halves so
            # mm1 chases each half; a_nat tiles queue after the whole aT
            # stream (their consumers run last).
            at_tiles = []
            for b in range(BPC):
                at_tiles.append(
                    atpool.tile([128, KD, TX], BF16, name=f"at{b}", tag="at")
                )
            an_tiles = []
            for i in range(KNAT):
                an_tiles.append(
                    anpool.tile([128, NT, DA], BF16, name=f"an{i}", tag="an")
                )

            nc.sync.dma_start(at_tiles[0][:, :, 0:1024], aT[0, :, :, 0:1024])

            w1a_sb = cpool.tile([128, KD, 64], BF16)
            nc.gpsimd.dma_start(w1a_sb[:], w1a[:])
            w1s_sb = cpool.tile([128, KD, H], F32)
            nc.gpsimd.dma_start(w1s_sb[:], w1s[:])
            sT_sb = cpool.tile([128, KD, BPC], F32)
            nc.gpsimd.dma_start(sT_sb[:], sT[:])
            b1c_sb = cpool.tile([128, 1], F32)
            nc.gpsimd.dma_start(b1c_sb[:], b1c[:])
            w2d_sb = cpool.tile([128, 2], BF16)
            nc.gpsimd.dma_start(w2d_sb[:], w2d[:])
            b2c_sb = cpool.tile([128, 1], F32)
            nc.gpsimd.dma_start(b2c_sb[:], b2c[:])
            ones_sb = cpool.tile([128, 128], BF16)
            nc.gpsimd.dma_start(ones_sb[:], ones[:])
            idm4_sb = cpool.tile([4, 4], BF16)
            nc.gpsimd.dma_start(idm4_sb[:], idm4[:])

            nc.sync.dma_start(at_tiles[0][:, :, 1024:2048], aT[0, :, :, 1024:2048])
            for b in range(1, BPC):
                nc.sync.dma_start(at_tiles[b][:, :, 0:1024], aT[b, :, :, 0:1024])
                nc.sync.dma_start(
                    at_tiles[b][:, :, 1024:2048], aT[b, :, :, 1024:2048]
                )
            for i in range(KNAT):
                nc.sync.dma_start(an_tiles[i][:, 0:8, :], a_nat[i, :, 0:8, :])
                nc.sync.dma_start(an_tiles[i][:, 8:16, :], a_nat[i, :, 8:16, :])

            sterm_sb = sb2.tile([128, BPC], F32)
            p_ab = sb2.tile([128, 2, 512], BF16)  # softmax rows, batches 0-3
            p_p4 = sb2.tile([128, 2, 512], BF16)  # softmax rows, batches 4-7
            pscr1 = sb2.tile([128, 2, 512], BF16)  # odd-parity vec rows, aligned
            pcomp0 = sb2.tile([4, 2, 512], BF16)  # nat rows, even parity
            pcomp1 = sb2.tile([4, 2, 512], BF16)  # nat rows, odd parity
            pT_sb = sb2.tile([128, NT, KNAT], BF16)
            sttout = sb2.tile([128, 512], BF16)  # stt elementwise dump
            ctxp_sb = sb2.tile([128, NVEC * KD * NQ], F32)
            ctxq_sb = sb2.tile([1, KNAT, DA], F32)
            den_sb = sb2.tile([128, NGRP, 2], F32)

            with tc.tile_pool(name="hps", bufs=2, space=PSUM) as hps, tc.tile_pool(
                name="eps", bufs=1, space=PSUM
            ) as eps, tc.tile_pool(
                name="pbc", bufs=2, space=PSUM
            ) as pbcp, tc.tile_pool(
                name="wps", bufs=1, space=PSUM
            ) as wpsp, tc.tile_pool(name="hsb", bufs=6) as hsbp, tc.tile_pool(
                name="tsb", bufs=2
            ) as tsbp:
                # PE warm-up: dense dummy matmuls on zeroed scratch keep the
                # PE busy during the initial DMA window (HAM p-state ramp).
                warm_sb = sb2.tile([128, 512], BF16, tag="warm")
                nc.vector.memset(warm_sb[:], 0.0)
                warm_ps = wpsp.tile([128, 512], F32, tag="wps", name="warm_ps")
                for _ in range(WARMUP):
                    nc.tensor.matmul(
                        warm_ps[0:64, :],
                        warm_sb[:, 0:64],
                        warm_sb[:],
                        start=True,
                        stop=True,
                        skip_group_check=True,
                    )
                # s-term, twice: partitions 0-49 (row group 0) and 64-113
                # (row group 64), so both relu halves get a bias.  Full
                # memset keeps rows 50-63/114-127 zero: mm2's 128-row
                # diagonal contraction touches them (against zero weights).
                nc.gpsimd.memset(sterm_sb[:], 0.0)
                nc.gpsimd.memset(den_sb[:], 0.0)
                sterm_ps = hps.tile([128, BPC], F32, tag="hps")
                for cg in (0, 64):
                    for k in range(KD):
                        nc.tensor.matmul(
                            sterm_ps[cg : cg + H, :],
                            w1s_sb[:, k, :],
                            sT_sb[:, k, :],
                            start=(k == 0),
                            stop=(k == KD - 1),
                            tile_position=(0, cg),
                            skip_group_check=True,
                        )
                    nc.scalar.activation(
                        sterm_sb[cg : cg + H, :],
                        sterm_ps[cg : cg + H, :],
                        AF.Identity,
                        bias=b1c_sb[cg : cg + H, :],
                    )

                # FIFO of deferred PE emitters spliced into later PE stream.
                # When it runs dry early on (DMA-chase phase), emit dummy
                # 1-column-stationary matmuls instead to hold the HAM clock
                # gate open.
                pending = []
                dummies = [0]

                def dummy():
                    if dummies[0] >= DUMMY_MAX:
                        return
                    dummies[0] += 1
                    nc.tensor.matmul(
                        warm_ps[0:1, :],
                        warm_sb[:, 0:1],
                        warm_sb[:],
                        start=True,
                        stop=True,
                        skip_group_check=True,
                    )

                def drain(n, fill=False):
                    for _ in range(n):
                        if pending:
                            pending.pop(0)()
                        elif fill:
                            dummy()
                        else:
                            return

                def emit_vec(b, q):
                    """Vector-route ctx quarter q for batch b: PE-broadcast
                    the p slice into PSUM, DVE fused mult+accum per k."""
                    tp, par = q // 2, q % 2
                    row = 32 * b
                    src = p_ab if par == 0 else pscr1

                    def emit():
                        pb = pbcp.tile([128, 512], F32, tag="pbc", name="pb")
                        nc.tensor.matmul(
                            pb[:],
                            ones_sb[row : row + 1, :],
                            src[row : row + 1, tp, :],
                            start=True,
                            stop=True,
                            tile_position=(row, 0),
                            skip_group_check=True,
                        )
                        for k in range(KD):
                            nc.vector.scalar_tensor_tensor(
                                out=sttout[:],
                                in0=at_tiles[b][:, k, 512 * q : 512 * (q + 1)],
                                scalar=1.0,
                                in1=pb[:],
                                op0=ALU.mult,
                                op1=ALU.mult,
                                accum_out=ctxp_sb[
                                    :, (b * KD + k) * NQ + q : (b * KD + k) * NQ + q + 1
                                ],
                            )

                    return emit

                def emit_softmax(e_t, p0, nrows, gi, p_tile):
                    for tp in range(2):
                        t_sb = tsbp.tile([128, 512], F32, tag="tsb")
                        nc.scalar.activation(
                            t_sb[p0 : p0 + nrows, :],
                            e_t[p0 : p0 + nrows, tp, :],
                            AF.Tanh,
                            bias=b2c_sb[p0 : p0 + nrows, :],
                        )
                        nc.scalar.activation(
                            p_tile[p0 : p0 + nrows, tp, :],
                            t_sb[p0 : p0 + nrows, :],
                            AF.Exp,
                            accum_out=den_sb[p0 : p0 + nrows, gi, tp : tp + 1],
                        )

                def emit_ptq(qgrp):
                    """Transpose p chunks 4*qgrp..4*qgrp+3 for all KNAT
                    batches at once (pcomp rows 0-3, time-contiguous)."""

                    def emit():
                        pt_ps = pbcp.tile(
                            [128, 4, KNAT], BF16, tag="pbc", name="pt"
                        )
                        for c in range(4):
                            n = 4 * qgrp + c
                            tp, par, off = n // 8, (n // 4) % 2, 128 * (n % 4)
                            pc = pcomp0 if par == 0 else pcomp1
                            nc.tensor.transpose(
                                pt_ps[:, c, :],
                                pc[0:KNAT, tp, off : off + 128],
                                idm4_sb[0:KNAT, 0:KNAT],
                            )
                        nc.vector.tensor_copy(
                            pT_sb[:, 4 * qgrp : 4 * qgrp + 4, :], pt_ps[:]
                        )

                    return emit

                def emit_quads(j, c_all, nlo, nhi):
                    def emit():
                        for n in range(nlo, nhi):
                            nc.tensor.matmul(
                                c_all[0:1, j, :],
                                pT_sb[:, n, j : j + 1],
                                an_tiles[j][:, n, :],
                                start=(n == 0),
                                stop=(n == NT - 1),
                                tile_position=(0, 0),
                                skip_group_check=True,
                            )

                    return emit

                e_ab = eps.tile([128, 2, 512], F32, tag="eps", name="e_ab")
                e_p4 = None
                c_all = None
                h_tiles = {}
                for b in range(BPC):
                    at_t = at_tiles[b]
                    if b == NVEC:
                        e_p4 = eps.tile([128, 2, 512], F32, tag="eps", name="e_p4")
                    R = 32 * (b % 4)
                    e_t = e_ab if b < NVEC else e_p4
                    for tp in range(2):
                        h_ps = hps.tile([128, 512], F32, tag="hps")
                        for k in range(KD):
                            for half, cg in enumerate((0, 64)):
                                ts = 2 * tp + half
                                nc.tensor.matmul(
                                    h_ps[cg : cg + 64, :],
                                    w1a_sb[:, k, :],
                                    at_t[:, k, ts * 512 : (ts + 1) * 512],
                                    start=(k == 0),
                                    stop=(k == KD - 1),
                                    tile_position=(0, cg),
                                    skip_group_check=True,
                                )
                        h_sb = hsbp.tile([128, 512], BF16, tag="hsb")
                        h_tiles[(b, tp)] = h_sb
                        nc.scalar.activation(
                            h_sb[:], h_ps[:], AF.Relu, bias=sterm_sb[:, b : b + 1]
                        )
                        drain(2, fill=(b < 6))
                    # mm2 diag: one 512-col matmul per tp gives both slices;
                    # e rows at partitions {R, R+1}.
                    for tp in range(2):
                        nc.tensor.matmul(
                            e_t[R : R + 2, tp, :],
                            w2d_sb[:],
                            h_tiles[(b, tp)][:],
                            start=True,
                            stop=True,
                            tile_position=(0, R),
                            skip_group_check=True,
                        )
                    drain(1, fill=(b < 6))
                    if b == 1 or b == 3:
                        p0 = 0 if b == 1 else 64
                        emit_softmax(e_ab, p0, 64, b // 2, p_ab)
                        # gather the two odd-parity rows to 32-aligned slots
                        nc.scalar.dma_start(
                            pscr1[p0 : p0 + 33 : 32, :, :],
                            p_ab[p0 + 1 : p0 + 34 : 32, :, :],
                        )
                        for bb in (b - 1, b):
                            for q in range(NQ):
                                pending.append(emit_vec(bb, q))
                    if b == BPC - 1:
                        emit_softmax(e_p4, 0, 128, 2, p_p4)
                        # gather p rows per parity to contiguous partitions
                        # 0-3 (scalar-engine HWDGE: separate FIFO from Sync).
                        nc.scalar.dma_start(pcomp0[0:KNAT, :, :], p_p4[0:98:32, :, :])
                        nc.scalar.dma_start(pcomp1[0:KNAT, :, :], p_p4[1:99:32, :, :])
                        c_all = eps.tile([1, KNAT, DA], F32, tag="eps", name="c_all")
                        for qgrp in range(4):
                            pending.append(emit_ptq(qgrp))
                        for j in range(KNAT):
                            for nlo in range(0, NT, 4):
                                pending.append(
                                    emit_quads(j, c_all, nlo, nlo + 4)
                                )

                drain(len(pending))
                # vec-route partials + denominators out (SWDGE, idle queue)
                nc.gpsimd.dma_start(ctxp_o[:], ctxp_sb[:])
                nc.gpsimd.dma_start(den_o[:], den_sb[:])
                # PE-route ctx: evacuate the accumulation row and ship last.
                nc.vector.tensor_copy(ctxq_sb[0:1, 0:2, :], c_all[0:1, 0:2, :])
                nc.scalar.activation(
                    ctxq_sb[0:1, 2:KNAT, :], c_all[0:1, 2:KNAT, :], AF.Identity
                )
                nc.sync.dma_start(ctxq_o[:], ctxq_sb[:])

    nc.compile()
    return nc


def make_in_maps(a, s, W1, b1, W2, b2):
    a = np.asarray(a, np.float32)
    s = np.asarray(s, np.float32)
    W1 = np.asarray(W1, np.float32)
    b1 = np.asarray(b1, np.float32)
    W2 = np.asarray(W2, np.float32)
    b2 = np.asarray(b2, np.float32)

    a5 = a.reshape(NCORES, BPC, TX, DA)
    s3 = s.reshape(NCORES, BPC, DS)

    w1a_h = np.zeros((128, KD, 64), np.float32)
    w1a_h[:, :, :H] = W1[:DA].reshape(KD, 128, H).transpose(1, 0, 2)
    w1a_h = w1a_h.astype(NPBF16)
    w1s_h = np.ascontiguousarray(
        W1[DA:].reshape(KD, 128, H).transpose(1, 0, 2)
    ).astype(np.float32)
    b1c_h = np.zeros((128, 1), np.float32)
    b1c_h[0:H, 0] = b1
    b1c_h[64 : 64 + H, 0] = b1
    w2d_h = np.zeros((128, 2), np.float32)
    w2d_h[0:H, 0] = W2[:, 0]
    w2d_h[64 : 64 + H, 1] = W2[:, 0]
    w2d_h = w2d_h.astype(NPBF16)
    b2c_h = np.full((128, 1), float(b2.reshape(-1)[0]), np.float32)
    ones_h = np.ones((128, 128), NPBF16)
    idm4_h = np.eye(4).astype(NPBF16)

    in_maps = []
    for i in range(NCORES):
        ai = a5[i]
        aT_h = np.ascontiguousarray(
            ai.transpose(0, 2, 1)
            .reshape(BPC, KD, 128, TX)
            .transpose(0, 2, 1, 3)
        ).astype(NPBF16)
        a_nat_h = np.ascontiguousarray(
            ai[NVEC:].reshape(KNAT, NT, 128, DA).transpose(0, 2, 1, 3)
        ).astype(NPBF16)
        sT_h = np.ascontiguousarray(
            s3[i].T.reshape(KD, 128, BPC).transpose(1, 0, 2)
        ).astype(np.float32)
        in_maps.append(
            {
                "aT": aT_h,
                "a_nat": a_nat_h,
                "w1a": w1a_h,
                "w1s": w1s_h,
                "sT": sT_h,
                "b1c": b1c_h,
                "w2d": w2d_h,
                "b2c": b2c_h,
                "ones": ones_h,
                "idm4": idm4_h,
            }
        )
    return in_maps


def assemble_output(results):
    outs = []
    for i in range(NCORES):
        r = results[i]
        ctxp = r["ctxp_o"].astype(np.float64).reshape(128, NVEC, KD, NQ)
        ctxq = r["ctxq_o"].astype(np.float64)  # [1, KNAT, DA]
        den3 = r["den_o"].astype(np.float64)  # [128, NGRP, 2]
        full = np.empty((BPC, DA), np.float64)
        for b in range(NVEC):
            full[b] = ctxp[:, b, :, :].sum(-1).T.reshape(DA)
        full[NVEC:] = ctxq[0]
        den = np.empty((BPC, 1), np.float64)
        for b in range(BPC):
            gi = b // 2 if b < NVEC else 2
            R = 32 * (b % 4)
            den[b, 0] = den3[R : R + 2, gi, :].sum()
        outs.append(full / den)
    return np.concatenate(outs, 0).reshape(B, 1, DA).astype(np.float32)


_NC_CACHE = None


def _get_nc():
    global _NC_CACHE
    if _NC_CACHE is None:
        _NC_CACHE = build_nc()
    return _NC_CACHE


def kernel(a, s, W1, b1, W2, b2, trace=False):
    from concourse.bass_utils import run_bass_kernel_spmd

    nc = _get_nc()
    in_maps = make_in_maps(a, s, W1, b1, W2, b2)
    res = run_bass_kernel_spmd(
        nc, in_maps, core_ids=list(range(NCORES)), trace=trace
    )
    out = assemble_output(res.results)
    if trace:
        kernel.last_exec_time_ns = res.exec_time_ns
        kernel.last_results = res
    return out


# revision 12
# speedup vs baseline: 1.0706x; 1.0547x over previous
"""Trainium2 Bass kernel for nn_AttentionLayer (Bahdanau-style attention scorer).

Math (per batch b):
    x   = concat([a, broadcast(s)], -1)            # [Tx, Da+Ds]
    h   = relu(x @ W1 + b1)                        # [Tx, H]
    e   = tanh(h @ W2 + b2)                        # [Tx, 1]
    al  = softmax(e, axis=Tx)
    ctx = al^T @ a                                 # [1, Da]

Since e = tanh(.) is in [-1, 1], softmax needs no max subtraction.

Sharding: data-parallel over B across 8 cores (8 batches each).

v5 design — tail-free streaming, 12.75 MB/core, diagonal-packed mm2:
`a` ships transposed+bf16 (aT, features on partitions) for ALL batches,
plus natural-layout (a_nat) bf16 for batches 4-7 only, queued after the
aT stream so each a_nat lands just-in-time for its ctx matmuls.

Scores: mm1 as column-tiled pairs (two 512-wide time slices through
array cols 0-63/64-127); relu + s-term bias on ACT.  mm2 uses a
block-diagonal stationary [128, 2] (W2 copy at rows 0-49 -> col 0, rows
64-113 -> col 1), so ONE 512-col matmul per (batch, tp) yields both
time slices: e rows land at PSUM partitions {R, R+1}, R = 32*(b%4).
e/p free layout is [tp, u]: global t = 1024*tp + 512*parity(row) + u.
Softmax groups: pairs {0,1} (rows 0-31..), {2,3} (rows 64..) sharing one
PSUM tile, and a page {4,5,6,7}.  tanh/exp on ACT with accum_out
partial denominators (summed on host; division on host).

Context:
  - batches 0-3 ("vector route"): PE broadcasts the batch's softmax row
    into PSUM quarters (ones-stationary matmul; odd-parity rows are
    first DMA-gathered to 32-aligned partitions since matmul operand
    base partitions must be 32-aligned); DVE runs the fused
    scalar_tensor_tensor (mult + accum_out) against aT slices.
  - batches 4-7 ("PE route"): p rows gathered to contiguous partitions
    0-3 per parity (scalar-engine HWDGE, partition-strided SBUF->SBUF),
    transposed time-major by PE (16 transposes cover all 4 batches),
    then ctx_j = sum_n pT_n^T @ a_nat_n as one accumulation chain per
    batch at PSUM partition 0.

HAM: the PE clock-gate un-throttles only under dense array activity, so
during the early DMA-chase phase (no deferred work available yet) the
drain() scheduler emits 1-column-stationary dummy matmuls to keep the
array streaming at full clock.

Host-side preprocessing (transpose/cast/shard + final division) is numpy.
"""

import os
import sys

import numpy as np

for _p in ("/opt/trn_rl_repo", "/root/.axon_site/_ro/trn_rl_repo"):
    if os.path.isdir(_p) and _p not in sys.path:
        sys.path.insert(0, _p)

import ml_dtypes  # noqa: E402

import concourse.bacc as bacc  # noqa: E402
import concourse.bass as bass  # noqa: E402
import concourse.mybir as mybir  # noqa: E402
import concourse.tile as tile  # noqa: E402

BF16 = mybir.dt.bfloat16
F32 = mybir.dt.float32
NPBF16 = ml_dtypes.bfloat16
AF = mybir.ActivationFunctionType
ALU = mybir.AluOpType
PSUM = bass.MemorySpace.PSUM

NCORES = 8
B, TX, DA, DS, H = 64, 2048, 256, 256, 50
BPC = B // NCORES  # batches per core
NT = TX // 128  # 128-wide time chunks (PE-route ctx)
KD = DA // 128  # feature chunks of a

NVEC = 4  # batches 0..NVEC-1 vector route; batches NVEC..7 PE route
KNAT = BPC - NVEC
NQ = 4  # ctx partial quarters per (batch, k) on the vector route
NGRP = 3  # softmax groups: {0,1}, {2,3}, {4,5,6,7}
WARMUP = 20
DUMMY_MAX = 150


def build_nc():
    """Build the (SPMD-identical) single-core Bass program."""
    nc = bacc.Bacc(
        "TRN2", target_bir_lowering=False, debug=False, num_devices=NCORES
    )

    aT = nc.dram_tensor("aT", [BPC, 128, KD, TX], BF16, kind="ExternalInput")
    a_nat = nc.dram_tensor(
        "a_nat", [KNAT, 128, NT, DA], BF16, kind="ExternalInput"
    )
    w1a = nc.dram_tensor("w1a", [128, KD, 64], BF16, kind="ExternalInput")
    w1s = nc.dram_tensor("w1s", [128, KD, H], F32, kind="ExternalInput")
    sT = nc.dram_tensor("sT", [128, KD, BPC], F32, kind="ExternalInput")
    # b1c carries the bias at partition rows 0-49 and 64-113 (the two
    # relu halves); w2d is the block-diagonal mm2 stationary.
    b1c = nc.dram_tensor("b1c", [128, 1], F32, kind="ExternalInput")
    w2d = nc.dram_tensor("w2d", [128, 2], BF16, kind="ExternalInput")
    b2c = nc.dram_tensor("b2c", [128, 1], F32, kind="ExternalInput")
    ones = nc.dram_tensor("ones", [128, 128], BF16, kind="ExternalInput")
    idm98 = nc.dram_tensor("idm98", [98, 98], BF16, kind="ExternalInput")
    ctxp_o = nc.dram_tensor(
        "ctxp_o", [128, NVEC * KD * NQ], F32, kind="ExternalOutput"
    )
    ctxq_o = nc.dram_tensor("ctxq_o", [1, KNAT, DA], F32, kind="ExternalOutput")
    den_o = nc.dram_tensor("den_o", [128, NGRP, 2], F32, kind="ExternalOutput")

    with tile.TileContext(nc) as tc:
        with tc.tile_pool(name="const", bufs=1) as cpool, tc.tile_pool(
            name="atp", bufs=BPC
        ) as atpool, tc.tile_pool(
            name="anp", bufs=KNAT
        ) as anpool, tc.tile_pool(name="sb2", bufs=1) as sb2:
            # DMA issue order is the schedule (one HWDGE FIFO on Sync):
            # aT tiles stream first in batch order, split in time-halves so
            # mm1 chases each half; a_nat tiles queue after the whole aT
            # stream (their consumers run last).
            at_tiles = []
            for b in range(BPC):
                at_tiles.append(
                    atpool.tile([128, KD, TX], BF16, name=f"at{b}", tag="at")
                )
            an_tiles = []
            for i in range(KNAT):
                an_tiles.append(
                    anpool.tile([128, NT, DA], BF16, name=f"an{i}", tag="an")
                )

            nc.sync.dma_start(at_tiles[0][:, :, 0:1024], aT[0, :, :, 0:1024])

            w1a_sb = cpool.tile([128, KD, 64], BF16)
            nc.gpsimd.dma_start(w1a_sb[:], w1a[:])
            w1s_sb = cpool.tile([128, KD, H], F32)
            nc.gpsimd.dma_start(w1s_sb[:], w1s[:])
            sT_sb = cpool.tile([128, KD, BPC], F32)
            nc.gpsimd.dma_start(sT_sb[:], sT[:])
            b1c_sb = cpool.tile([128, 1], F32)
            nc.gpsimd.dma_start(b1c_sb[:], b1c[:])
            w2d_sb = cpool.tile([128, 2], BF16)
            nc.gpsimd.dma_start(w2d_sb[:], w2d[:])
            b2c_sb = cpool.tile([128, 1], F32)
            nc.gpsimd.dma_start(b2c_sb[:], b2c[:])
            ones_sb = cpool.tile([128, 128], BF16)
            nc.gpsimd.dma_start(ones_sb[:], ones[:])
            idm98_sb = cpool.tile([98, 98], BF16)
            nc.gpsimd.dma_start(idm98_sb[:], idm98[:])

            nc.sync.dma_start(at_tiles[0][:, :, 1024:2048], aT[0, :, :, 1024:2048])
            for b in range(1, BPC):
                nc.sync.dma_start(at_tiles[b][:, :, 0:1024], aT[b, :, :, 0:1024])
                nc.sync.dma_start(
                    at_tiles[b][:, :, 1024:2048], aT[b, :, :, 1024:2048]
                )
            for i in range(KNAT):
                nc.sync.dma_start(an_tiles[i][:, 0:8, :], a_nat[i, :, 0:8, :])
                nc.sync.dma_start(an_tiles[i][:, 8:16, :], a_nat[i, :, 8:16, :])

            sterm_sb = sb2.tile([128, BPC], F32)
            p_ab = sb2.tile([128, 2, 512], BF16)  # softmax rows, batches 0-3
            p_p4 = sb2.tile([128, 2, 512], BF16)  # softmax rows, batches 4-7
            pscr1 = sb2.tile([128, 2, 512], BF16)  # odd-parity vec rows, aligned
            pT_sb = sb2.tile([128, NT, KNAT], BF16)
            sttout = sb2.tile([128, 512], BF16)  # stt elementwise dump
            ctxp_sb = sb2.tile([128, NVEC * KD * NQ], F32)
            ctxq_sb = sb2.tile([1, KNAT, DA], F32)
            den_sb = sb2.tile([128, NGRP, 2], F32)

            with tc.tile_pool(name="hps", bufs=2, space=PSUM) as hps, tc.tile_pool(
                name="eps", bufs=1, space=PSUM
            ) as eps, tc.tile_pool(
                name="pbc", bufs=2, space=PSUM
            ) as pbcp, tc.tile_pool(
                name="wps", bufs=1, space=PSUM
            ) as wpsp, tc.tile_pool(name="hsb", bufs=6) as hsbp, tc.tile_pool(
                name="tsb", bufs=2
            ) as tsbp:
                # PE warm-up: dense dummy matmuls on zeroed scratch keep the
                # PE busy during the initial DMA window (HAM p-state ramp).
                warm_sb = sb2.tile([128, 512], BF16, tag="warm")
                nc.vector.memset(warm_sb[:], 0.0)
                warm_ps = wpsp.tile([128, 512], F32, tag="wps", name="warm_ps")
                for _ in range(WARMUP):
                    nc.tensor.matmul(
                        warm_ps[0:64, :],
                        warm_sb[:, 0:64],
                        warm_sb[:],
                        start=True,
                        stop=True,
                        skip_group_check=True,
                    )
                # s-term, twice: partitions 0-49 (row group 0) and 64-113
                # (row group 64), so both relu halves get a bias.  Full
                # memset keeps rows 50-63/114-127 zero: mm2's 128-row
                # diagonal contraction touches them (against zero weights).
                nc.gpsimd.memset(sterm_sb[:], 0.0)
                nc.gpsimd.memset(den_sb[:], 0.0)
                sterm_ps = hps.tile([128, BPC], F32, tag="hps")
                for cg in (0, 64):
                    for k in range(KD):
                        nc.tensor.matmul(
                            sterm_ps[cg : cg + H, :],
                            w1s_sb[:, k, :],
                            sT_sb[:, k, :],
                            start=(k == 0),
                            stop=(k == KD - 1),
                            tile_position=(0, cg),
                            skip_group_check=True,
                        )
                    nc.scalar.activation(
                        sterm_sb[cg : cg + H, :],
                        sterm_ps[cg : cg + H, :],
                        AF.Identity,
                        bias=b1c_sb[cg : cg + H, :],
                    )

                # FIFO of deferred PE emitters spliced into later PE stream.
                # When it runs dry early on (DMA-chase phase), emit dummy
                # 1-column-stationary matmuls instead to hold the HAM clock
                # gate open.
                pending = []
                dummies = [0]

                def dummy():
                    if dummies[0] >= DUMMY_MAX:
                        return
                    dummies[0] += 1
                    nc.tensor.matmul(
                        warm_ps[0:1, :],
                        warm_sb[:, 0:1],
                        warm_sb[:],
                        start=True,
                        stop=True,
                        skip_group_check=True,
                    )

                def drain(n, fill=False):
                    for _ in range(n):
                        if pending:
                            pending.pop(0)()
                        elif fill:
                            dummy()
                        else:
                            return

                def emit_vec(b, q):
                    """Vector-route ctx quarter q for batch b: PE-broadcast
                    the p slice into PSUM, DVE fused mult+accum per k."""
                    tp, par = q // 2, q % 2
                    row = 32 * b
                    src = p_ab if par == 0 else pscr1

                    def emit():
                        pb = pbcp.tile([128, 512], F32, tag="pbc", name="pb")
                        nc.tensor.matmul(
                            pb[:],
                            ones_sb[row : row + 1, :],
                            src[row : row + 1, tp, :],
                            start=True,
                            stop=True,
                            tile_position=(row, 0),
                            skip_group_check=True,
                        )
                        for k in range(KD):
                            nc.vector.scalar_tensor_tensor(
                                out=sttout[:],
                                in0=at_tiles[b][:, k, 512 * q : 512 * (q + 1)],
                                scalar=1.0,
                                in1=pb[:],
                                op0=ALU.mult,
                                op1=ALU.mult,
                                accum_out=ctxp_sb[
                                    :, (b * KD + k) * NQ + q : (b * KD + k) * NQ + q + 1
                                ],
                            )

                    return emit

                def emit_softmax(e_t, p0, nrows, gi, p_tile):
                    for tp in range(2):
                        t_sb = tsbp.tile([128, 512], F32, tag="tsb")
                        nc.scalar.activation(
                            t_sb[p0 : p0 + nrows, :],
                            e_t[p0 : p0 + nrows, tp, :],
                            AF.Tanh,
                            bias=b2c_sb[p0 : p0 + nrows, :],
                        )
                        nc.scalar.activation(
                            p_tile[p0 : p0 + nrows, tp, :],
                            t_sb[p0 : p0 + nrows, :],
                            AF.Exp,
                            accum_out=den_sb[p0 : p0 + nrows, gi, tp : tp + 1],
                        )

                def emit_ptq(tp):
                    """Transpose p rows 0-97 directly from p_p4 for time
                    half tp: 4 transposes of [98, 128] cover chunks
                    8*tp..8*tp+7 (both parities — strided copies pick the
                    batch rows per parity)."""

                    def emit():
                        pt_ps = pbcp.tile(
                            [128, 4, 98], BF16, tag="pbc", name="pt"
                        )
                        for c in range(4):
                            nc.tensor.transpose(
                                pt_ps[:, c, :],
                                p_p4[0:98, tp, 128 * c : 128 * (c + 1)],
                                idm98_sb[:],
                            )
                        for par in range(2):
                            nc.vector.tensor_copy(
                                pT_sb[:, 8 * tp + 4 * par : 8 * tp + 4 * par + 4, :],
                                pt_ps[:, :, par : par + 97 : 32],
                            )

                    return emit

                def emit_quads(j, c_all, nlo, nhi):
                    def emit():
                        for n in range(nlo, nhi):
                            nc.tensor.matmul(
                                c_all[0:1, j, :],
                                pT_sb[:, n, j : j + 1],
                                an_tiles[j][:, n, :],
                                start=(n == 0),
                                stop=(n == NT - 1),
                                tile_position=(0, 0),
                                skip_group_check=True,
                            )

                    return emit

                e_ab = eps.tile([128, 2, 512], F32, tag="eps", name="e_ab")
                e_p4 = None
                c_all = None
                h_tiles = {}
                for b in range(BPC):
                    at_t = at_tiles[b]
                    if b == NVEC:
                        e_p4 = eps.tile([128, 2, 512], F32, tag="eps", name="e_p4")
                    R = 32 * (b % 4)
                    e_t = e_ab if b < NVEC else e_p4
                    # mm1 k-pass: same stationary for consecutive matmuls
                    # (slices ts and ts+2 share (k, cg)); both h_ps tiles
                    # accumulate across the k passes, relus at batch end.
                    h_ps2 = [
                        hps.tile([128, 512], F32, tag="hps", name=f"hp{b}_{t_}")
                        for t_ in range(2)
                    ]
                    for k in range(KD):
                        for half, cg in enumerate((0, 64)):
                            for tp in range(2):
                                ts = 2 * tp + half
                                nc.tensor.matmul(
                                    h_ps2[tp][cg : cg + 64, :],
                                    w1a_sb[:, k, :],
                                    at_t[:, k, ts * 512 : (ts + 1) * 512],
                                    start=(k == 0),
                                    stop=(k == KD - 1),
                                    tile_position=(0, cg),
                                    skip_group_check=True,
                                )
                            drain(1, fill=True)
                    for tp in range(2):
                        h_sb = hsbp.tile([128, 512], BF16, tag="hsb")
                        h_tiles[(b, tp)] = h_sb
                        nc.scalar.activation(
                            h_sb[:],
                            h_ps2[tp][:],
                            AF.Relu,
                            bias=sterm_sb[:, b : b + 1],
                        )
                        drain(1, fill=True)
                    # mm2 diag: one 512-col matmul per tp gives both slices;
                    # e rows at partitions {R, R+1}.
                    for tp in range(2):
                        nc.tensor.matmul(
                            e_t[R : R + 2, tp, :],
                            w2d_sb[:],
                            h_tiles[(b, tp)][:],
                            start=True,
                            stop=True,
                            tile_position=(0, R),
                            skip_group_check=True,
                        )
                    drain(1, fill=True)
                    if b == 1 or b == 3:
                        p0 = 0 if b == 1 else 64
                        emit_softmax(e_ab, p0, 64, b // 2, p_ab)
                        # gather the two odd-parity rows to 32-aligned slots
                        nc.scalar.dma_start(
                            pscr1[p0 : p0 + 33 : 32, :, :],
                            p_ab[p0 + 1 : p0 + 34 : 32, :, :],
                        )
                        for bb in (b - 1, b):
                            for q in range(NQ):
                                pending.append(emit_vec(bb, q))
                    if b == BPC - 1:
                        emit_softmax(e_p4, 0, 128, 2, p_p4)
                        c_all = eps.tile([1, KNAT, DA], F32, tag="eps", name="c_all")
                        for tp in range(2):
                            pending.append(emit_ptq(tp))
                        for j in range(KNAT):
                            for nlo in range(0, NT, 4):
                                pending.append(
                                    emit_quads(j, c_all, nlo, nlo + 4)
                                )

                drain(len(pending))
                # vec-route partials + denominators out (SWDGE, idle queue)
                nc.gpsimd.dma_start(ctxp_o[:], ctxp_sb[:])
                nc.gpsimd.dma_start(den_o[:], den_sb[:])
                # PE-route ctx: evacuate the accumulation row and ship last.
                nc.vector.tensor_copy(ctxq_sb[0:1, 0:2, :], c_all[0:1, 0:2, :])
                nc.scalar.activation(
                    ctxq_sb[0:1, 2:KNAT, :], c_all[0:1, 2:KNAT, :], AF.Identity
                )
                nc.sync.dma_start(ctxq_o[:], ctxq_sb[:])

    nc.compile()
    return nc


def make_in_maps(a, s, W1, b1, W2, b2):
    a = np.asarray(a, np.float32)
    s = np.asarray(s, np.float32)
    W1 = np.asarray(W1, np.float32)
    b1 = np.asarray(b1, np.float32)
    W2 = np.asarray(W2, np.float32)
    b2 = np.asarray(b2, np.float32)

    a5 = a.reshape(NCORES, BPC, TX, DA)
    s3 = s.reshape(NCORES, BPC, DS)

    w1a_h = np.zeros((128, KD, 64), np.float32)
    w1a_h[:, :, :H] = W1[:DA].reshape(KD, 128, H).transpose(1, 0, 2)
    w1a_h = w1a_h.astype(NPBF16)
    w1s_h = np.ascontiguousarray(
        W1[DA:].reshape(KD, 128, H).transpose(1, 0, 2)
    ).astype(np.float32)
    b1c_h = np.zeros((128, 1), np.float32)
    b1c_h[0:H, 0] = b1
    b1c_h[64 : 64 + H, 0] = b1
    w2d_h = np.zeros((128, 2), np.float32)
    w2d_h[0:H, 0] = W2[:, 0]
    w2d_h[64 : 64 + H, 1] = W2[:, 0]
    w2d_h = w2d_h.astype(NPBF16)
    b2c_h = np.full((128, 1), float(b2.reshape(-1)[0]), np.float32)
    ones_h = np.ones((128, 128), NPBF16)
    idm98_h = np.eye(98).astype(NPBF16)

    in_maps = []
    for i in range(NCORES):
        ai = a5[i]
        aT_h = np.ascontiguousarray(
            ai.transpose(0, 2, 1)
            .reshape(BPC, KD, 128, TX)
            .transpose(0, 2, 1, 3)
        ).astype(NPBF16)
        a_nat_h = np.ascontiguousarray(
            ai[NVEC:].reshape(KNAT, NT, 128, DA).transpose(0, 2, 1, 3)
        ).astype(NPBF16)
        sT_h = np.ascontiguousarray(
            s3[i].T.reshape(KD, 128, BPC).transpose(1, 0, 2)
        ).astype(np.float32)
        in_maps.append(
            {
                "aT": aT_h,
                "a_nat": a_nat_h,
                "w1a": w1a_h,
                "w1s": w1s_h,
                "sT": sT_h,
                "b1c": b1c_h,
                "w2d": w2d_h,
                "b2c": b2c_h,
                "ones": ones_h,
                "idm98": idm98_h,
            }
        )
    return in_maps


def assemble_output(results):
    outs = []
    for i in range(NCORES):
        r = results[i]
        ctxp = r["ctxp_o"].astype(np.float64).reshape(128, NVEC, KD, NQ)
        ctxq = r["ctxq_o"].astype(np.float64)  # [1, KNAT, DA]
        den3 = r["den_o"].astype(np.float64)  # [128, NGRP, 2]
        full = np.empty((BPC, DA), np.float64)
        for b in range(NVEC):
            full[b] = ctxp[:, b, :, :].sum(-1).T.reshape(DA)
        full[NVEC:] = ctxq[0]
        den = np.empty((BPC, 1), np.float64)
        for b in range(BPC):
            gi = b // 2 if b < NVEC else 2
            R = 32 * (b % 4)
            den[b, 0] = den3[R : R + 2, gi, :].sum()
        outs.append(full / den)
    return np.concatenate(outs, 0).reshape(B, 1, DA).astype(np.float32)


_NC_CACHE = None


def _get_nc():
    global _NC_CACHE
    if _NC_CACHE is None:
        _NC_CACHE = build_nc()
    return _NC_CACHE


def kernel(a, s, W1, b1, W2, b2, trace=False):
    from concourse.bass_utils import run_bass_kernel_spmd

    nc = _get_nc()
    in_maps = make_in_maps(a, s, W1, b1, W2, b2)
    res = run_bass_kernel_spmd(
        nc, in_maps, core_ids=list(range(NCORES)), trace=trace
    )
    out = assemble_output(res.results)
    if trace:
        kernel.last_exec_time_ns = res.exec_time_ns
        kernel.last_results = res
    return out
